# revision 25
# baseline (speedup 1.0000x reference)
"""Trainium2 Bass kernel for nn_BinaryClassifier (CNN + 2-qubit circuit head).

Data-parallel over 8 cores (65536 -> 8192/core), NT=1024 images per tile.

Structure per tile:
  conv1: 12 blocks (4y x 12x out, patch 8x16 = K 128 full), 1 matmul each.
         Bias folded into the PSUM drain (tensor_scalar w/ AP bias).
  pool1: drain -> stage1 max(yp halves, cross-partition-offset TT)
         -> stage2 max+relu (STT) writing 32-row chunks into conv2 K-tiles.
  conv2: 8 out-tiles x 2 accumulated matmuls over [128,N] K-tiles
         (zero weight rows where a row is outside the 6x8 patch).
  pool2: same pattern -> fc1 rhs tiles F2a/F2b.
  fc1:   2 matmuls + Act relu/bias drain.  fc2: 8 small matmuls.
  head:  quantum circuit reduced to 6 cosines, computed once after the loop.
"""
import os, sys
sys.path.insert(0, "/opt/trn_rl_repo")
import numpy as np
import ml_dtypes

from concourse import bass, tile, bacc
from concourse import mybir
from concourse.bass_utils import run_bass_kernel_spmd

dt = mybir.dt
AF = mybir.ActivationFunctionType
ALU = mybir.AluOpType

B = int(os.environ.get("BASS_KERNEL_B", "65536"))
NCORES = int(os.environ.get("BASS_KERNEL_CORES", "8"))
BC = B // NCORES          # images per core
NT = min(1024, BC)        # images per tile
NTILES = BC // NT
NSUB = NT // 128          # fc2 image-subtiles per tile

# conv1 blocking: 12 blocks = 6 by (4 out rows each) x 2 bx (12 out cols each)
# patch per block: 8 rows x 16 cols = 128 K-rows (full)
# M layout: quad (yp, xq) * 32 + payload; payload = pc*4 + pr*2 + ch (24 used)


def _bf16(a):
    return np.asarray(a, dtype=np.float32).astype(np.float16)


# ---------------------------------------------------------------- host packing

def build_a(x):
    """x: [B, 784] -> [12, 128, B] bf16 im2col (block, patch-pixel, image)."""
    n = x.shape[0]
    xb = _bf16(x).reshape(n, 28, 28)
    out = np.empty((12, 128, n), dtype=np.float16)
    for by in range(6):
        for bx in range(2):
            b = by * 2 + bx
            for iy in range(8):
                src = xb[:, 4 * by + iy, 12 * bx:12 * bx + 16]   # [n,16]
                out[b, iy * 16:iy * 16 + 16, :] = src.T
    return out


def _m_decode(m):
    """conv1 M index -> (valid, dy, dx, ch): out pixel local coords."""
    quad, pay = m >> 5, m & 31
    if pay >= 24:
        return False, 0, 0, 0
    yp, xq = quad >> 1, quad & 1
    pc, r = pay >> 2, pay & 3
    pr, ch = r >> 1, r & 1
    return True, 2 * pr + yp, 2 * pc + xq, ch


def build_w1(w1):
    """w1: [2,1,5,5] -> [128, 12*128] bf16 (same lhsT for every block)."""
    W = np.zeros((128, 128), dtype=np.float32)
    w1 = np.asarray(w1, dtype=np.float32).reshape(2, 5, 5)
    for m in range(128):
        ok, dy, dx, ch = _m_decode(m)
        if not ok:
            continue
        for ky in range(5):
            for kx in range(5):
                W[(dy + ky) * 16 + (dx + kx), m] = w1[ch, ky, kx]
    return _bf16(W)


def build_w2(w2):
    """w2: [16,2,5,5] -> [128, 16*128] bf16.

    mm i = j*2 + bxi, j = (a, xh) = 2*a + xh.  K-tile (yh(j), bxi):
    K-row k = 32*slot + pc*4 + pr*2 + ch  (slot = chunk index 0..3)
      chunk by = slot + 2*yh;  pooled R = 2*by + pr, C = 6*bxi + pc.
    M col m = quad(yp2, xp2)*32 + oc*2 + bb:
      out pixel oy = 2*a + yp2, ox = 4*xh + 2*bb + xp2.
    """
    W = np.zeros((128, 16 * 128), dtype=np.float32)
    w2 = np.asarray(w2, dtype=np.float32)
    for a in range(4):
        yh = 0 if a < 2 else 1
        for xh in range(2):
            j = 2 * a + xh
            for bxi in range(2):
                col0 = (j * 2 + bxi) * 128
                for slot in range(4):
                    by = slot + 2 * yh
                    for pc in range(6):
                        for pr in range(2):
                            for ch in range(2):
                                k = 32 * slot + pc * 4 + pr * 2 + ch
                                R = 2 * by + pr
                                C = 6 * bxi + pc
                                for m in range(128):
                                    quad, pay = m >> 5, m & 31
                                    yp2, xp2 = quad >> 1, quad & 1
                                    oc, bb = pay >> 1, pay & 1
                                    oy = 2 * a + yp2
                                    ox = 4 * xh + 2 * bb + xp2
                                    ky, kx = R - oy, C - ox
                                    if 0 <= ky < 5 and 0 <= kx < 5:
                                        W[k, col0 + m] = w2[oc, ch, ky, kx]
    return _bf16(W)


def build_wfc1(fc1_w):
    """fc1_w: [64, 256] -> [128, 2*64] bf16 (2 K-tiles F2a/F2b)."""
    W = np.zeros((128, 2 * 64), dtype=np.float32)
    fc1_w = np.asarray(fc1_w, dtype=np.float32)
    for t in range(2):
        for p in range(128):
            jl, pay = p >> 5, p & 31
            j = 4 * t + jl
            a, xh = j >> 1, j & 1
            oc, bb = pay >> 1, pay & 1
            b_ = 2 * xh + bb
            flat = oc * 16 + a * 4 + b_
            W[p, t * 64:t * 64 + 64] = fc1_w[:, flat]
    return _bf16(W)


def head_constants(qnn_params, fc3_w, fc3_b, fc2_b):
    """Reduce the 2-qubit circuit tail + fc3 to z = c0 + sum Mk*cos(...)."""
    p = np.asarray(qnn_params, dtype=np.float64)

    def ry(t):
        c, s = np.cos(t), np.sin(t)
        return np.array([[c, -s], [s, c]])

    def kron_w0(U):
        return np.kron(U, np.eye(2))

    def kron_w1(U):
        return np.kron(np.eye(2), U)

    CN01 = np.zeros((4, 4)); CN01[0, 0] = CN01[1, 1] = 1; CN01[2, 3] = CN01[3, 2] = 1
    CN10 = np.zeros((4, 4))
    for q0 in range(2):
        for q1 in range(2):
            CN10[((q0 ^ q1) * 2 + q1), q0 * 2 + q1] = 1
    U = np.eye(4)
    U = kron_w0(ry(p[0])) @ U
    U = kron_w1(ry(p[1])) @ U
    U = CN01 @ U
    U = kron_w0(ry(p[2])) @ U
    U = kron_w1(ry(p[3])) @ U
    U = CN10 @ U
    U = kron_w0(ry(p[4])) @ U
    U = kron_w1(ry(p[5])) @ U
    U = CN01 @ U
    U = kron_w0(ry(p[6])) @ U
    U = kron_w1(ry(p[7])) @ U
    S = np.diag([1.0, -1.0, -1.0, 1.0])
    M = 0.25 * (U.T @ S @ U)
    w3 = float(np.asarray(fc3_w).reshape(()))
    b3 = float(np.asarray(fc3_b).reshape(()))
    c0 = float(np.trace(M)) * w3 + b3
    k = {
        "A": 2 * M[0, 3] * w3,   # cos(2x0+2x1)
        "B": 2 * M[1, 2] * w3,   # cos(2x0-2x1)
        "C": 2 * M[0, 2] * w3,   # cos(2x0+2ang)
        "D": 2 * M[1, 3] * w3,   # cos(2x0-2ang)
        "E": 2 * M[0, 1] * w3,   # cos(2x1+2ang)
        "F": 2 * M[2, 3] * w3,   # cos(2x1-2ang)
    }
    return c0, k, float(fc2_b[0]), float(fc2_b[1])


# ---------------------------------------------------------------- bass program

def build_program(weights):
    nc = bacc.Bacc(None, target_bir_lowering=False, debug=False)
    a_d = nc.declare_dram_parameter("a_c1", [12, 128, BC], dt.float16, isOutput=False)
    w1_d = nc.declare_dram_parameter("w1", [128, 128], dt.float16, isOutput=False)
    w2_d = nc.declare_dram_parameter("w2", [128, 16 * 128], dt.float16, isOutput=False)
    wf1_d = nc.declare_dram_parameter("wf1", [128, 2 * 64], dt.float16, isOutput=False)
    wf2_d = nc.declare_dram_parameter("wf2", [64, 2], dt.float16, isOutput=False)
    cst_d = nc.declare_dram_parameter("cst", [128, 16], dt.float32, isOutput=False)
    y_d = nc.declare_dram_parameter("y", [2, BC], dt.float32, isOutput=True)

    c0, K, b20, b21 = weights["head"]
    pi = float(np.pi)

    with tile.TileContext(nc) as tc:
        with tc.tile_pool(name="cw", bufs=1) as cw, \
             tc.tile_pool(name="sx", bufs=2) as sx, \
             tc.tile_pool(name="sc", bufs=3) as sc, \
             tc.tile_pool(name="sm", bufs=2) as sm, \
             tc.tile_pool(name="se", bufs=2) as se, \
             tc.tile_pool(name="sf", bufs=2) as sf, \
             tc.tile_pool(name="hd", bufs=1) as hd, \
             tc.tile_pool(name="p1", bufs=2, space="PSUM") as p1:

            W1 = cw.tile([128, 128], dt.float16)
            nc.sync.dma_start(out=W1[:], in_=w1_d[:])
            W2 = cw.tile([128, 16 * 128], dt.float16)
            nc.sync.dma_start(out=W2[:], in_=w2_d[:])
            WF1 = cw.tile([128, 2 * 64], dt.float16)
            nc.sync.dma_start(out=WF1[:], in_=wf1_d[:])
            WF2 = cw.tile([64, 2], dt.float16)
            nc.sync.dma_start(out=WF2[:], in_=wf2_d[:])
            CST = cw.tile([128, 16], dt.float32)
            nc.sync.dma_start(out=CST[:], in_=cst_d[:])
            Hall = cw.tile([128, NTILES * 2 * NSUB], dt.float32)

            # engine rotation: drains on DVE/Act (GpSimd cannot read PSUM),
            # SBUF-only pool stages on DVE/GpSimd
            drain_eng = [nc.vector, nc.scalar]
            tt_eng = [nc.vector, nc.vector]
            cp_eng = nc.gpsimd

            for it in range(NTILES):
                n0 = it * NT
                xc = sx.tile([128, 12 * NT], dt.float16)
                nc.sync.dma_start(
                    out=xc[:].rearrange("p (b n) -> p b n", b=12),
                    in_=a_d[:, :, n0:n0 + NT].transpose([1, 0, 2]))

                # T K-tiles for conv2: [yh] each [128, 2*NT], free = (bx, n)
                T = [se.tile([128, 2 * NT], dt.float16, tag=f"T{yh}",
                             name=f"T{yh}") for yh in range(2)]

                di = 0
                ti = 0
                # conv1: block pairs (by, bx=0/1) share weights -> one matmul
                # of N=2*NT into a 4-bank psum tile
                for by in range(6):
                    ps = p1.tile([128, 2 * NT], dt.float32, tag="ps")
                    for q in range(2 * NT // 512):
                        c0_ = 2 * by * NT + q * 512
                        nc.tensor.matmul(out=ps[:, q * 512:(q + 1) * 512],
                                         lhsT=W1[:, 0:128],
                                         rhs=xc[:, c0_:c0_ + 512],
                                         start=True, stop=True)
                    # drain + conv1 bias (per-partition AP scalar)
                    C = sc.tile([128, 2 * NT], dt.float16, tag="C")
                    eng = drain_eng[di % 2]; di += 1
                    if eng is nc.scalar:
                        nc.scalar.activation(out=C[:], in_=ps[:], func=AF.Relu,
                                             bias=CST[:, 0:1])
                    else:
                        eng.tensor_scalar(out=C[:], in0=ps[:],
                                          scalar1=CST[:, 0:1], scalar2=None,
                                          op0=ALU.add)
                    # stage1: max over yp; upper half copied to base-0 first
                    # (two-input SBUF ops require equal base partitions)
                    S1 = sm.tile([64, 2 * NT], dt.float16, tag="S1")
                    cp_eng.tensor_copy(out=S1[:], in_=C[64:128, :])
                    M1 = sm.tile([64, 2 * NT], dt.float16, tag="M1")
                    eng = tt_eng[ti % 2]; ti += 1
                    eng.tensor_tensor(out=M1[:], in0=C[0:64, :], in1=S1[:],
                                      op=ALU.max)
                    S2 = sm.tile([32, 2 * NT], dt.float16, tag="S2")
                    cp_eng.tensor_copy(out=S2[:], in_=M1[32:64, :])
                    # stage2: max over xq + relu -> chunk row of T (both blocks)
                    for yh in range(2):
                        slot = by - 2 * yh
                        if 0 <= slot < 4:
                            eng = tt_eng[ti % 2]; ti += 1
                            eng.scalar_tensor_tensor(
                                out=T[yh][32 * slot:32 * slot + 32, :],
                                in0=M1[0:32, :], scalar=0.0, in1=S2[:],
                                op0=ALU.max, op1=ALU.max)

                # conv2: 8 out-tiles x 2 accumulated matmuls; pairs (j, j+1)
                # share one 4-bank psum tile and batched drain/stage1
                F2 = [sf.tile([128, NT], dt.float16, tag=f"F2{t}", name=f"F2{t}")
                      for t in range(2)]
                for a_ in range(4):
                    yh = 0 if a_ < 2 else 1
                    ps2 = p1.tile([128, 2 * NT], dt.float32, tag="ps")
                    for xh in range(2):
                        j = 2 * a_ + xh
                        for bxi in range(2):
                            for q in range(NT // 512):
                                nc.tensor.matmul(
                                    out=ps2[:, xh * NT + q * 512:xh * NT + (q + 1) * 512],
                                    lhsT=W2[:, (j * 2 + bxi) * 128:(j * 2 + bxi + 1) * 128],
                                    rhs=T[yh][:, bxi * NT + q * 512:bxi * NT + (q + 1) * 512],
                                    start=(bxi == 0), stop=(bxi == 1))
                    D = sc.tile([128, 2 * NT], dt.float16, tag="D")
                    eng = drain_eng[di % 2]; di += 1
                    if eng is nc.scalar:
                        nc.scalar.activation(out=D[:], in_=ps2[:], func=AF.Relu,
                                             bias=CST[:, 1:2])
                    else:
                        eng.tensor_scalar(out=D[:], in0=ps2[:],
                                          scalar1=CST[:, 1:2], scalar2=None,
                                          op0=ALU.add)
                    S1b = sm.tile([64, 2 * NT], dt.float16, tag="S1b")
                    cp_eng.tensor_copy(out=S1b[:], in_=D[64:128, :])
                    M2 = sm.tile([64, 2 * NT], dt.float16, tag="M2")
                    eng = tt_eng[ti % 2]; ti += 1
                    eng.tensor_tensor(out=M2[:], in0=D[0:64, :], in1=S1b[:],
                                      op=ALU.max)
                    S2b = sm.tile([32, 2 * NT], dt.float16, tag="S2b")
                    cp_eng.tensor_copy(out=S2b[:], in_=M2[32:64, :])
                    for xh in range(2):
                        j = 2 * a_ + xh
                        ft, jl = j // 4, j % 4
                        eng = tt_eng[ti % 2]; ti += 1
                        eng.scalar_tensor_tensor(
                            out=F2[ft][32 * jl:32 * jl + 32, :],
                            in0=M2[0:32, xh * NT:(xh + 1) * NT], scalar=0.0,
                            in1=S2b[:, xh * NT:(xh + 1) * NT],
                            op0=ALU.max, op1=ALU.max)

                # fc1 (K=256 over 2 tiles) -> relu -> F1 bf16
                psf = p1.tile([64, 2 * NT], dt.float32, tag="ps")
                for t_ in range(2):
                    for q in range(NT // 512):
                        nc.tensor.matmul(out=psf[:, q * 512:(q + 1) * 512],
                                         lhsT=WF1[:, 64 * t_:64 * (t_ + 1)],
                                         rhs=F2[t_][:, q * 512:(q + 1) * 512],
                                         start=(t_ == 0), stop=(t_ == 1))
                F1 = sf.tile([64, NT], dt.float16, tag="F1")
                nc.scalar.activation(out=F1[:], in_=psf[:, 0:NT], func=AF.Relu,
                                     bias=CST[0:64, 2:3])

                # fc2 img-major: NSUB matmuls N=2 -> psum [128, 2*NSUB]
                psg = p1.tile([128, 2 * NT], dt.float32, tag="ps")
                for s in range(NSUB):
                    nc.tensor.matmul(out=psg[:, 2 * s:2 * s + 2],
                                     lhsT=F1[:, 128 * s:128 * (s + 1)],
                                     rhs=WF2[:], start=True, stop=True)
                nc.vector.tensor_copy(out=Hall[:, it * 2 * NSUB:(it + 1) * 2 * NSUB],
                                      in_=psg[:, 0:2 * NSUB])

            # ---- head once: Hall [128, (t, s, c)] -> Yall [128, (t, s)]
            NC_ = NTILES * NSUB       # head column count
            Hv = Hall[:].rearrange("p (u c) -> p u c", c=2)
            x0 = Hv[:, :, 0]
            x1 = Hv[:, :, 1]
            t0 = hd.tile([128, NC_], dt.float32)
            nc.vector.tensor_scalar(out=t0[:], in0=x0, scalar1=-1.0,
                                    scalar2=pi - b20, op0=ALU.mult, op1=ALU.add)
            t1 = hd.tile([128, NC_], dt.float32)
            nc.gpsimd.tensor_scalar(out=t1[:], in0=x1, scalar1=-1.0,
                                    scalar2=pi - b21, op0=ALU.mult, op1=ALU.add)
            ang = hd.tile([128, NC_], dt.float32)
            nc.vector.tensor_tensor(out=ang[:], in0=t0[:], in1=t1[:], op=ALU.mult)

            qpi = pi / 4
            hb = {"A": b20 + b21 + qpi, "B": b20 - b21 + qpi,
                  "C": b20 + qpi, "D": b20 + qpi,
                  "E": b21 + qpi, "F": b21 + qpi}
            AR = hd.tile([128, 6 * NC_], dt.float32)
            plan = (("A", x0, x1, ALU.add), ("B", x0, x1, ALU.subtract),
                    ("C", x0, ang[:], ALU.add), ("D", x0, ang[:], ALU.subtract),
                    ("E", x1, ang[:], ALU.add), ("F", x1, ang[:], ALU.subtract))
            for i, (nm, a0, a1, op) in enumerate(plan):
                eng = tt_eng[i % 2]
                eng.scalar_tensor_tensor(
                    out=AR[:, NC_ * i:NC_ * (i + 1)], in0=a0, scalar=hb[nm],
                    in1=a1, op0=ALU.add, op1=op)
            # range reduce: h = AR - pi*round(AR/pi); sin(2h) = sin(2AR mod 2pi)
            tq = hd.tile([128, 6 * NC_], dt.float32)
            nc.vector.tensor_scalar(out=tq[:], in0=AR[:], scalar1=float(1 / pi),
                                    scalar2=None, op0=ALU.mult)
            ti_ = hd.tile([128, 6 * NC_], dt.int32)
            nc.vector.tensor_copy(out=ti_[:], in_=tq[:])
            tf_ = hd.tile([128, 6 * NC_], dt.float32)
            nc.vector.tensor_copy(out=tf_[:], in_=ti_[:])
            hh = hd.tile([128, 6 * NC_], dt.float32)
            nc.vector.scalar_tensor_tensor(out=hh[:], in0=tf_[:], scalar=-pi,
                                           in1=AR[:], op0=ALU.mult, op1=ALU.add)
            SN = hd.tile([128, 6 * NC_], dt.float32)
            nc.scalar.activation(out=SN[:], in_=hh[:], func=AF.Sin, scale=2.0)
            cosv = {nm: SN[:, NC_ * i:NC_ * (i + 1)]
                    for i, nm in enumerate("ABCDEF")}

            acc = hd.tile([128, NC_], dt.float32, tag="acc0")
            nc.vector.tensor_scalar(out=acc[:], in0=cosv["A"], scalar1=K["A"],
                                    scalar2=c0, op0=ALU.mult, op1=ALU.add)
            for i, nm in enumerate("BCDE"):
                acc2 = hd.tile([128, NC_], dt.float32, tag=f"acc{i+1}")
                eng = tt_eng[i % 2]
                eng.scalar_tensor_tensor(out=acc2[:], in0=cosv[nm][:],
                                         scalar=K[nm], in1=acc[:],
                                         op0=ALU.mult, op1=ALU.add)
                acc = acc2
            Yall = hd.tile([128, NC_], dt.float32, tag="Yall")
            nc.vector.scalar_tensor_tensor(out=Yall[:], in0=cosv["F"],
                                           scalar=K["F"], in1=acc[:],
                                           op0=ALU.mult, op1=ALU.add)

            # ---- final: out0=-ln(1+e^{1-2y}), out1=-ln(1+e^{2y-1})
            V = hd.tile([128, NC_], dt.float32)
            nc.scalar.activation(out=V[:], in_=Yall[:], func=AF.Exp,
                                 bias=CST[:, 8:9], scale=-2.0)      # e^{1-2y}
            Wr = hd.tile([128, NC_], dt.float32)
            nc.vector.reciprocal(out=Wr[:], in_=V[:])               # e^{2y-1}
            L0 = hd.tile([128, NC_], dt.float32)
            nc.scalar.activation(out=L0[:], in_=V[:], func=AF.Ln,
                                 bias=CST[:, 8:9], scale=1.0)       # ln(1+v)
            L1 = hd.tile([128, NC_], dt.float32)
            nc.scalar.activation(out=L1[:], in_=Wr[:], func=AF.Ln,
                                 bias=CST[:, 8:9], scale=1.0)
            O = hd.tile([128, 2 * NC_], dt.float32)
            Ov = O[:].rearrange("p (c u) -> p c u", c=2)
            nc.vector.tensor_scalar(out=Ov[:, 0, :], in0=L0[:], scalar1=-1.0,
                                    scalar2=None, op0=ALU.mult)
            nc.gpsimd.tensor_scalar(out=Ov[:, 1, :], in0=L1[:], scalar1=-1.0,
                                    scalar2=None, op0=ALU.mult)
            # y layout [2, BC]: dst[c, it*NT + s*128 + p] <- O[p, (c, it, s)]
            for c in range(2):
                nc.sync.dma_start(
                    out=y_d[c, :].rearrange("(t s p) -> p t s", p=128, s=NSUB),
                    in_=Ov[:, c, :].rearrange("p (t s) -> p t s", s=NSUB))

    nc.compile()
    return nc


def kernel(x, conv1_w, conv1_b, conv2_w, conv2_b, fc1_w, fc1_b,
           fc2_w, fc2_b, fc3_w, fc3_b, qnn_params):
    x = np.asarray(x, dtype=np.float32).reshape(B, 784)
    a = build_a(x)
    W1 = build_w1(conv1_w)
    W2 = build_w2(conv2_w)
    WF1 = build_wfc1(fc1_w)
    WF2 = _bf16(np.asarray(fc2_w, np.float32).T)  # [64, 2]
    c0, K, b20, b21 = head_constants(qnn_params, fc3_w, fc3_b,
                                     np.asarray(fc2_b, np.float32))
    cst = np.zeros((128, 16), dtype=np.float32)
    b1 = np.asarray(conv1_b, np.float32)
    b2 = np.asarray(conv2_b, np.float32)
    for p in range(128):
        pay = p & 31
        cst[p, 0] = b1[pay & 1] if pay < 24 else 0.0   # conv1 bias (ch = p&1)
        cst[p, 1] = b2[pay >> 1]                        # conv2 bias (oc)
    cst[0:64, 2] = np.asarray(fc1_b, np.float32)
    cst[:, 8] = 1.0

    weights = {"head": (c0, K, b20, b21)}
    nc = build_program(weights)

    in_maps = []
    for c in range(NCORES):
        sl = slice(c * BC, (c + 1) * BC)
        in_maps.append({
            "a_c1": np.ascontiguousarray(a[:, :, sl]),
            "w1": W1, "w2": W2, "wf1": WF1, "wf2": WF2, "cst": cst,
        })
    res = run_bass_kernel_spmd(nc, in_maps, list(range(NCORES)),
                               trace=bool(int(os.environ.get("BASS_TRACE_KERNEL", "0"))))
    if res.exec_time_ns is not None:
        print(f"HW exec time: {res.exec_time_ns} ns")
    global LAST_RESULTS
    LAST_RESULTS = res.results
    out = np.empty((B, 2), dtype=np.float32)
    for c in range(NCORES):
        out[c * BC:(c + 1) * BC] = res.results[c]["y"].T
    return out


# revision 26
# speedup vs baseline: 2.5879x; 2.5879x over previous
"""Trainium2 Bass kernel for nn_BinaryClassifier (CNN + 2-qubit circuit head).

Data-parallel over 8 cores (65536 -> 8192/core), NT=1024 images per tile.

Structure per tile:
  conv1: 12 blocks (4y x 12x out, patch 8x16 = K 128 full), 1 matmul each.
         Bias folded into the PSUM drain (tensor_scalar w/ AP bias).
  pool1: drain -> stage1 max(yp halves, cross-partition-offset TT)
         -> stage2 max+relu (STT) writing 32-row chunks into conv2 K-tiles.
  conv2: 8 out-tiles x 2 accumulated matmuls over [128,N] K-tiles
         (zero weight rows where a row is outside the 6x8 patch).
  pool2: same pattern -> fc1 rhs tiles F2a/F2b.
  fc1:   2 matmuls + Act relu/bias drain.  fc2: 8 small matmuls.
  head:  quantum circuit reduced to 6 cosines, computed once after the loop.
"""
import os, sys
sys.path.insert(0, "/opt/trn_rl_repo")
import numpy as np
import ml_dtypes

from concourse import bass, tile, bacc
from concourse import mybir
from concourse.bass_utils import run_bass_kernel_spmd

dt = mybir.dt
AF = mybir.ActivationFunctionType
ALU = mybir.AluOpType

B = int(os.environ.get("BASS_KERNEL_B", "65536"))
NCORES = int(os.environ.get("BASS_KERNEL_CORES", "8"))
BC = B // NCORES          # images per core
NT = min(1024, BC)        # images per tile
NTILES = BC // NT
NSUB = NT // 128          # fc2 image-subtiles per tile

# conv1 blocking: 12 blocks = 6 by (4 out rows each) x 2 bx (12 out cols each)
# patch per block: 8 rows x 16 cols = 128 K-rows (full)
# M layout: quad (yp, xq) * 32 + payload; payload = pc*4 + pr*2 + ch (24 used)


def _bf16(a):
    return np.asarray(a, dtype=np.float32).astype(np.float16)


# ---------------------------------------------------------------- host packing

def build_a(x):
    """x: [B, 784] -> [12, 128, B] bf16 im2col (block, patch-pixel, image)."""
    n = x.shape[0]
    xb = _bf16(x).reshape(n, 28, 28)
    out = np.empty((12, 128, n), dtype=np.float16)
    for by in range(6):
        for bx in range(2):
            b = by * 2 + bx
            for iy in range(8):
                src = xb[:, 4 * by + iy, 12 * bx:12 * bx + 16]   # [n,16]
                out[b, iy * 16:iy * 16 + 16, :] = src.T
    return out


def _m_decode(m):
    """conv1 M index -> (valid, dy, dx, ch): out pixel local coords."""
    quad, pay = m >> 5, m & 31
    if pay >= 24:
        return False, 0, 0, 0
    yp, xq = quad >> 1, quad & 1
    pc, r = pay >> 2, pay & 3
    pr, ch = r >> 1, r & 1
    return True, 2 * pr + yp, 2 * pc + xq, ch


def build_w1(w1):
    """w1: [2,1,5,5] -> [128, 12*128] bf16 (same lhsT for every block)."""
    W = np.zeros((128, 128), dtype=np.float32)
    w1 = np.asarray(w1, dtype=np.float32).reshape(2, 5, 5)
    for m in range(128):
        ok, dy, dx, ch = _m_decode(m)
        if not ok:
            continue
        for ky in range(5):
            for kx in range(5):
                W[(dy + ky) * 16 + (dx + kx), m] = w1[ch, ky, kx]
    return _bf16(W)


def build_w2(w2):
    """w2: [16,2,5,5] -> [128, 16*128] bf16.

    mm i = j*2 + bxi, j = (a, xh) = 2*a + xh.  K-tile (yh(j), bxi):
    K-row k = 32*slot + pc*4 + pr*2 + ch  (slot = chunk index 0..3)
      chunk by = slot + 2*yh;  pooled R = 2*by + pr, C = 6*bxi + pc.
    M col m = quad(yp2, xp2)*32 + oc*2 + bb:
      out pixel oy = 2*a + yp2, ox = 4*xh + 2*bb + xp2.
    """
    W = np.zeros((128, 16 * 128), dtype=np.float32)
    w2 = np.asarray(w2, dtype=np.float32)
    for a in range(4):
        yh = 0 if a < 2 else 1
        for xh in range(2):
            j = 2 * a + xh
            for bxi in range(2):
                col0 = (j * 2 + bxi) * 128
                for slot in range(4):
                    by = slot + 2 * yh
                    for pc in range(6):
                        for pr in range(2):
                            for ch in range(2):
                                k = 32 * slot + pc * 4 + pr * 2 + ch
                                R = 2 * by + pr
                                C = 6 * bxi + pc
                                for m in range(128):
                                    quad, pay = m >> 5, m & 31
                                    yp2, xp2 = quad >> 1, quad & 1
                                    oc, bb = pay >> 1, pay & 1
                                    oy = 2 * a + yp2
                                    ox = 4 * xh + 2 * bb + xp2
                                    ky, kx = R - oy, C - ox
                                    if 0 <= ky < 5 and 0 <= kx < 5:
                                        W[k, col0 + m] = w2[oc, ch, ky, kx]
    return _bf16(W)


def build_wfc1(fc1_w):
    """fc1_w: [64, 256] -> [128, 2*64] bf16 (2 K-tiles F2a/F2b)."""
    W = np.zeros((128, 2 * 64), dtype=np.float32)
    fc1_w = np.asarray(fc1_w, dtype=np.float32)
    for t in range(2):
        for p in range(128):
            jl, pay = p >> 5, p & 31
            j = 4 * t + jl
            a, xh = j >> 1, j & 1
            oc, bb = pay >> 1, pay & 1
            b_ = 2 * xh + bb
            flat = oc * 16 + a * 4 + b_
            W[p, t * 64:t * 64 + 64] = fc1_w[:, flat]
    return _bf16(W)


def head_constants(qnn_params, fc3_w, fc3_b, fc2_b):
    """Reduce the 2-qubit circuit tail + fc3 to z = c0 + sum Mk*cos(...)."""
    p = np.asarray(qnn_params, dtype=np.float64)

    def ry(t):
        c, s = np.cos(t), np.sin(t)
        return np.array([[c, -s], [s, c]])

    def kron_w0(U):
        return np.kron(U, np.eye(2))

    def kron_w1(U):
        return np.kron(np.eye(2), U)

    CN01 = np.zeros((4, 4)); CN01[0, 0] = CN01[1, 1] = 1; CN01[2, 3] = CN01[3, 2] = 1
    CN10 = np.zeros((4, 4))
    for q0 in range(2):
        for q1 in range(2):
            CN10[((q0 ^ q1) * 2 + q1), q0 * 2 + q1] = 1
    U = np.eye(4)
    U = kron_w0(ry(p[0])) @ U
    U = kron_w1(ry(p[1])) @ U
    U = CN01 @ U
    U = kron_w0(ry(p[2])) @ U
    U = kron_w1(ry(p[3])) @ U
    U = CN10 @ U
    U = kron_w0(ry(p[4])) @ U
    U = kron_w1(ry(p[5])) @ U
    U = CN01 @ U
    U = kron_w0(ry(p[6])) @ U
    U = kron_w1(ry(p[7])) @ U
    S = np.diag([1.0, -1.0, -1.0, 1.0])
    M = 0.25 * (U.T @ S @ U)
    w3 = float(np.asarray(fc3_w).reshape(()))
    b3 = float(np.asarray(fc3_b).reshape(()))
    c0 = float(np.trace(M)) * w3 + b3
    k = {
        "A": 2 * M[0, 3] * w3,   # cos(2x0+2x1)
        "B": 2 * M[1, 2] * w3,   # cos(2x0-2x1)
        "C": 2 * M[0, 2] * w3,   # cos(2x0+2ang)
        "D": 2 * M[1, 3] * w3,   # cos(2x0-2ang)
        "E": 2 * M[0, 1] * w3,   # cos(2x1+2ang)
        "F": 2 * M[2, 3] * w3,   # cos(2x1-2ang)
    }
    return c0, k, float(fc2_b[0]), float(fc2_b[1])


# ---------------------------------------------------------------- bass program

def build_program(weights):
    nc = bacc.Bacc(None, target_bir_lowering=False, debug=False)
    a_d = nc.declare_dram_parameter("a_c1", [12, 128, BC], dt.float16, isOutput=False)
    w1_d = nc.declare_dram_parameter("w1", [128, 128], dt.float16, isOutput=False)
    w2_d = nc.declare_dram_parameter("w2", [128, 16 * 128], dt.float16, isOutput=False)
    wf1_d = nc.declare_dram_parameter("wf1", [128, 2 * 64], dt.float16, isOutput=False)
    wf2_d = nc.declare_dram_parameter("wf2", [64, 2], dt.float16, isOutput=False)
    cst_d = nc.declare_dram_parameter("cst", [128, 16], dt.float32, isOutput=False)
    y_d = nc.declare_dram_parameter("y", [2, BC], dt.float32, isOutput=True)

    c0, K, b20, b21 = weights["head"]
    pi = float(np.pi)

    with tile.TileContext(nc) as tc:
        with tc.tile_pool(name="cw", bufs=1) as cw, \
             tc.tile_pool(name="sx", bufs=2) as sx, \
             tc.tile_pool(name="sc", bufs=3) as sc, \
             tc.tile_pool(name="sm", bufs=2) as sm, \
             tc.tile_pool(name="se", bufs=2) as se, \
             tc.tile_pool(name="sf", bufs=2) as sf, \
             tc.tile_pool(name="hd", bufs=1) as hd, \
             tc.tile_pool(name="p1", bufs=2, space="PSUM") as p1:

            W1 = cw.tile([128, 128], dt.float16)
            nc.sync.dma_start(out=W1[:], in_=w1_d[:])
            W2 = cw.tile([128, 16 * 128], dt.float16)
            nc.sync.dma_start(out=W2[:], in_=w2_d[:])
            WF1 = cw.tile([128, 2 * 64], dt.float16)
            nc.sync.dma_start(out=WF1[:], in_=wf1_d[:])
            WF2 = cw.tile([64, 2], dt.float16)
            nc.sync.dma_start(out=WF2[:], in_=wf2_d[:])
            CST = cw.tile([128, 16], dt.float32)
            nc.sync.dma_start(out=CST[:], in_=cst_d[:])
            Hall = cw.tile([128, NTILES * 2 * NSUB], dt.float32)

            # engine rotation: drains on DVE/Act (GpSimd cannot read PSUM),
            # SBUF-only pool stages on DVE/GpSimd
            drain_eng = [nc.scalar, nc.scalar]
            tt_eng = [nc.vector, nc.vector]

            for it in range(NTILES):
                n0 = it * NT
                xc = sx.tile([128, 12 * NT], dt.float16)
                nc.sync.dma_start(
                    out=xc[:].rearrange("p (b n) -> p b n", b=12),
                    in_=a_d[:, :, n0:n0 + NT].transpose([1, 0, 2]))

                # T K-tiles for conv2: [yh] each [128, 2*NT], free = (bx, n)
                T = [se.tile([128, 2 * NT], dt.float16, tag=f"T{yh}",
                             name=f"T{yh}") for yh in range(2)]

                di = 0
                ti = 0
                # conv1: block pairs (by, bx=0/1) share weights -> one matmul
                # of N=2*NT into a 4-bank psum tile
                for by in range(6):
                    ps = p1.tile([128, 2 * NT], dt.float32, tag="ps")
                    for q in range(2 * NT // 512):
                        c0_ = 2 * by * NT + q * 512
                        nc.tensor.matmul(out=ps[:, q * 512:(q + 1) * 512],
                                         lhsT=W1[:, 0:128],
                                         rhs=xc[:, c0_:c0_ + 512],
                                         start=True, stop=True)
                    # drain + conv1 bias (per-partition AP scalar)
                    C = sc.tile([128, 2 * NT], dt.float16, tag="C")
                    eng = drain_eng[di % 2]; di += 1
                    if eng is nc.scalar:
                        nc.scalar.activation(out=C[:], in_=ps[:], func=AF.Relu,
                                             bias=CST[:, 0:1])
                    else:
                        eng.tensor_scalar(out=C[:], in0=ps[:],
                                          scalar1=CST[:, 0:1], scalar2=None,
                                          op0=ALU.add)
                    # stage1: max over yp; upper half copied to base-0 first
                    # (two-input SBUF ops require equal base partitions)
                    S1 = sm.tile([64, 2 * NT], dt.float16, tag="S1")
                    nc.sync.dma_start(out=S1[:], in_=C[64:128, :])
                    M1 = sm.tile([64, 2 * NT], dt.float16, tag="M1")
                    eng = tt_eng[ti % 2]; ti += 1
                    eng.tensor_tensor(out=M1[:], in0=C[0:64, :], in1=S1[:],
                                      op=ALU.max)
                    S2 = sm.tile([32, 2 * NT], dt.float16, tag="S2")
                    nc.sync.dma_start(out=S2[:], in_=M1[32:64, :])
                    # stage2: max over xq + relu -> chunk row of T (both blocks)
                    yh = 0 if by < 4 else 1
                    slot = by - 2 * yh
                    eng = tt_eng[ti % 2]; ti += 1
                    eng.scalar_tensor_tensor(
                        out=T[yh][32 * slot:32 * slot + 32, :],
                        in0=M1[0:32, :], scalar=0.0, in1=S2[:],
                        op0=ALU.max, op1=ALU.max)

                # chunks by=2,3 (T0 slots 2,3) also open T1 as slots 0,1
                nc.sync.dma_start(out=T[1][0:64, :], in_=T[0][64:128, :])

                # conv2: 8 out-tiles x 2 accumulated matmuls; pairs (j, j+1)
                # share one 4-bank psum tile and batched drain/stage1
                F2 = [sf.tile([128, NT], dt.float16, tag=f"F2{t}", name=f"F2{t}")
                      for t in range(2)]
                for a_ in range(4):
                    yh = 0 if a_ < 2 else 1
                    ps2 = p1.tile([128, 2 * NT], dt.float32, tag="ps")
                    for xh in range(2):
                        j = 2 * a_ + xh
                        for bxi in range(2):
                            for q in range(NT // 512):
                                nc.tensor.matmul(
                                    out=ps2[:, xh * NT + q * 512:xh * NT + (q + 1) * 512],
                                    lhsT=W2[:, (j * 2 + bxi) * 128:(j * 2 + bxi + 1) * 128],
                                    rhs=T[yh][:, bxi * NT + q * 512:bxi * NT + (q + 1) * 512],
                                    start=(bxi == 0), stop=(bxi == 1))
                    D = sc.tile([128, 2 * NT], dt.float16, tag="D")
                    eng = drain_eng[di % 2]; di += 1
                    if eng is nc.scalar:
                        nc.scalar.activation(out=D[:], in_=ps2[:], func=AF.Relu,
                                             bias=CST[:, 1:2])
                    else:
                        eng.tensor_scalar(out=D[:], in0=ps2[:],
                                          scalar1=CST[:, 1:2], scalar2=None,
                                          op0=ALU.add)
                    S1b = sm.tile([64, 2 * NT], dt.float16, tag="S1b")
                    nc.sync.dma_start(out=S1b[:], in_=D[64:128, :])
                    M2 = sm.tile([64, 2 * NT], dt.float16, tag="M2")
                    eng = tt_eng[ti % 2]; ti += 1
                    eng.tensor_tensor(out=M2[:], in0=D[0:64, :], in1=S1b[:],
                                      op=ALU.max)
                    S2b = sm.tile([32, 2 * NT], dt.float16, tag="S2b")
                    nc.sync.dma_start(out=S2b[:], in_=M2[32:64, :])
                    for xh in range(2):
                        j = 2 * a_ + xh
                        ft, jl = j // 4, j % 4
                        eng = tt_eng[ti % 2]; ti += 1
                        eng.scalar_tensor_tensor(
                            out=F2[ft][32 * jl:32 * jl + 32, :],
                            in0=M2[0:32, xh * NT:(xh + 1) * NT], scalar=0.0,
                            in1=S2b[:, xh * NT:(xh + 1) * NT],
                            op0=ALU.max, op1=ALU.max)

                # fc1 (K=256 over 2 tiles) -> relu -> F1 bf16
                psf = p1.tile([64, 2 * NT], dt.float32, tag="ps")
                for t_ in range(2):
                    for q in range(NT // 512):
                        nc.tensor.matmul(out=psf[:, q * 512:(q + 1) * 512],
                                         lhsT=WF1[:, 64 * t_:64 * (t_ + 1)],
                                         rhs=F2[t_][:, q * 512:(q + 1) * 512],
                                         start=(t_ == 0), stop=(t_ == 1))
                F1 = sf.tile([64, NT], dt.float16, tag="F1")
                nc.scalar.activation(out=F1[:], in_=psf[:, 0:NT], func=AF.Relu,
                                     bias=CST[0:64, 2:3])

                # fc2 img-major: NSUB matmuls N=2 -> psum [128, 2*NSUB]
                psg = p1.tile([128, 2 * NT], dt.float32, tag="ps")
                for s in range(NSUB):
                    nc.tensor.matmul(out=psg[:, 2 * s:2 * s + 2],
                                     lhsT=F1[:, 128 * s:128 * (s + 1)],
                                     rhs=WF2[:], start=True, stop=True)
                nc.vector.tensor_copy(out=Hall[:, it * 2 * NSUB:(it + 1) * 2 * NSUB],
                                      in_=psg[:, 0:2 * NSUB])

            # ---- head once: Hall [128, (t, s, c)] -> Yall [128, (t, s)]
            NC_ = NTILES * NSUB       # head column count
            Hv = Hall[:].rearrange("p (u c) -> p u c", c=2)
            x0 = Hv[:, :, 0]
            x1 = Hv[:, :, 1]
            t0 = hd.tile([128, NC_], dt.float32)
            nc.vector.tensor_scalar(out=t0[:], in0=x0, scalar1=-1.0,
                                    scalar2=pi - b20, op0=ALU.mult, op1=ALU.add)
            t1 = hd.tile([128, NC_], dt.float32)
            nc.vector.tensor_scalar(out=t1[:], in0=x1, scalar1=-1.0,
                                    scalar2=pi - b21, op0=ALU.mult, op1=ALU.add)
            ang = hd.tile([128, NC_], dt.float32)
            nc.vector.tensor_tensor(out=ang[:], in0=t0[:], in1=t1[:], op=ALU.mult)

            qpi = pi / 4
            hb = {"A": b20 + b21 + qpi, "B": b20 - b21 + qpi,
                  "C": b20 + qpi, "D": b20 + qpi,
                  "E": b21 + qpi, "F": b21 + qpi}
            AR = hd.tile([128, 6 * NC_], dt.float32)
            plan = (("A", x0, x1, ALU.add), ("B", x0, x1, ALU.subtract),
                    ("C", x0, ang[:], ALU.add), ("D", x0, ang[:], ALU.subtract),
                    ("E", x1, ang[:], ALU.add), ("F", x1, ang[:], ALU.subtract))
            for i, (nm, a0, a1, op) in enumerate(plan):
                eng = tt_eng[i % 2]
                eng.scalar_tensor_tensor(
                    out=AR[:, NC_ * i:NC_ * (i + 1)], in0=a0, scalar=hb[nm],
                    in1=a1, op0=ALU.add, op1=op)
            # range reduce: h = AR - pi*round(AR/pi); sin(2h) = sin(2AR mod 2pi)
            tq = hd.tile([128, 6 * NC_], dt.float32)
            nc.vector.tensor_scalar(out=tq[:], in0=AR[:], scalar1=float(1 / pi),
                                    scalar2=None, op0=ALU.mult)
            ti_ = hd.tile([128, 6 * NC_], dt.int32)
            nc.vector.tensor_copy(out=ti_[:], in_=tq[:])
            tf_ = hd.tile([128, 6 * NC_], dt.float32)
            nc.vector.tensor_copy(out=tf_[:], in_=ti_[:])
            hh = hd.tile([128, 6 * NC_], dt.float32)
            nc.vector.scalar_tensor_tensor(out=hh[:], in0=tf_[:], scalar=-pi,
                                           in1=AR[:], op0=ALU.mult, op1=ALU.add)
            SN = hd.tile([128, 6 * NC_], dt.float32)
            nc.scalar.activation(out=SN[:], in_=hh[:], func=AF.Sin, scale=2.0)
            cosv = {nm: SN[:, NC_ * i:NC_ * (i + 1)]
                    for i, nm in enumerate("ABCDEF")}

            acc = hd.tile([128, NC_], dt.float32, tag="acc0")
            nc.vector.tensor_scalar(out=acc[:], in0=cosv["A"], scalar1=K["A"],
                                    scalar2=c0, op0=ALU.mult, op1=ALU.add)
            for i, nm in enumerate("BCDE"):
                acc2 = hd.tile([128, NC_], dt.float32, tag=f"acc{i+1}")
                eng = tt_eng[i % 2]
                eng.scalar_tensor_tensor(out=acc2[:], in0=cosv[nm][:],
                                         scalar=K[nm], in1=acc[:],
                                         op0=ALU.mult, op1=ALU.add)
                acc = acc2
            Yall = hd.tile([128, NC_], dt.float32, tag="Yall")
            nc.vector.scalar_tensor_tensor(out=Yall[:], in0=cosv["F"],
                                           scalar=K["F"], in1=acc[:],
                                           op0=ALU.mult, op1=ALU.add)

            # ---- final: out0=-ln(1+e^{1-2y}), out1=-ln(1+e^{2y-1})
            V = hd.tile([128, NC_], dt.float32)
            nc.scalar.activation(out=V[:], in_=Yall[:], func=AF.Exp,
                                 bias=CST[:, 8:9], scale=-2.0)      # e^{1-2y}
            Wr = hd.tile([128, NC_], dt.float32)
            nc.vector.reciprocal(out=Wr[:], in_=V[:])               # e^{2y-1}
            L0 = hd.tile([128, NC_], dt.float32)
            nc.scalar.activation(out=L0[:], in_=V[:], func=AF.Ln,
                                 bias=CST[:, 8:9], scale=1.0)       # ln(1+v)
            L1 = hd.tile([128, NC_], dt.float32)
            nc.scalar.activation(out=L1[:], in_=Wr[:], func=AF.Ln,
                                 bias=CST[:, 8:9], scale=1.0)
            O = hd.tile([128, 2 * NC_], dt.float32)
            Ov = O[:].rearrange("p (c u) -> p c u", c=2)
            nc.vector.tensor_scalar(out=Ov[:, 0, :], in0=L0[:], scalar1=-1.0,
                                    scalar2=None, op0=ALU.mult)
            nc.vector.tensor_scalar(out=Ov[:, 1, :], in0=L1[:], scalar1=-1.0,
                                    scalar2=None, op0=ALU.mult)
            # y layout [2, BC]: dst[c, it*NT + s*128 + p] <- O[p, (c, it, s)]
            for c in range(2):
                nc.sync.dma_start(
                    out=y_d[c, :].rearrange("(t s p) -> p t s", p=128, s=NSUB),
                    in_=Ov[:, c, :].rearrange("p (t s) -> p t s", s=NSUB))

    nc.compile()
    return nc


def kernel(x, conv1_w, conv1_b, conv2_w, conv2_b, fc1_w, fc1_b,
           fc2_w, fc2_b, fc3_w, fc3_b, qnn_params):
    x = np.asarray(x, dtype=np.float32).reshape(B, 784)
    a = build_a(x)
    W1 = build_w1(conv1_w)
    W2 = build_w2(conv2_w)
    WF1 = build_wfc1(fc1_w)
    WF2 = _bf16(np.asarray(fc2_w, np.float32).T)  # [64, 2]
    c0, K, b20, b21 = head_constants(qnn_params, fc3_w, fc3_b,
                                     np.asarray(fc2_b, np.float32))
    cst = np.zeros((128, 16), dtype=np.float32)
    b1 = np.asarray(conv1_b, np.float32)
    b2 = np.asarray(conv2_b, np.float32)
    for p in range(128):
        pay = p & 31
        cst[p, 0] = b1[pay & 1] if pay < 24 else 0.0   # conv1 bias (ch = p&1)
        cst[p, 1] = b2[pay >> 1]                        # conv2 bias (oc)
    cst[0:64, 2] = np.asarray(fc1_b, np.float32)
    cst[:, 8] = 1.0

    weights = {"head": (c0, K, b20, b21)}
    nc = build_program(weights)

    in_maps = []
    for c in range(NCORES):
        sl = slice(c * BC, (c + 1) * BC)
        in_maps.append({
            "a_c1": np.ascontiguousarray(a[:, :, sl]),
            "w1": W1, "w2": W2, "wf1": WF1, "wf2": WF2, "cst": cst,
        })
    res = run_bass_kernel_spmd(nc, in_maps, list(range(NCORES)),
                               trace=bool(int(os.environ.get("BASS_TRACE_KERNEL", "0"))))
    if res.exec_time_ns is not None:
        print(f"HW exec time: {res.exec_time_ns} ns")
    global LAST_RESULTS
    LAST_RESULTS = res.results
    out = np.empty((B, 2), dtype=np.float32)
    for c in range(NCORES):
        out[c * BC:(c + 1) * BC] = res.results[c]["y"].T
    return out


# revision 27
# speedup vs baseline: 3.2513x; 1.2564x over previous
"""Trainium2 Bass kernel for nn_BinaryClassifier (CNN + 2-qubit circuit head).

Data-parallel over 8 cores (65536 -> 8192/core), NT=1024 images per tile.

Structure per tile:
  conv1: 12 blocks (4y x 12x out, patch 8x16 = K 128 full), 1 matmul each.
         Bias folded into the PSUM drain (tensor_scalar w/ AP bias).
  pool1: drain -> stage1 max(yp halves, cross-partition-offset TT)
         -> stage2 max+relu (STT) writing 32-row chunks into conv2 K-tiles.
  conv2: 8 out-tiles x 2 accumulated matmuls over [128,N] K-tiles
         (zero weight rows where a row is outside the 6x8 patch).
  pool2: same pattern -> fc1 rhs tiles F2a/F2b.
  fc1:   2 matmuls + Act relu/bias drain.  fc2: 8 small matmuls.
  head:  quantum circuit reduced to 6 cosines, computed once after the loop.
"""
import os, sys
sys.path.insert(0, "/opt/trn_rl_repo")
import numpy as np
import ml_dtypes

from concourse import bass, tile, bacc
from concourse import mybir
from concourse.bass_utils import run_bass_kernel_spmd

dt = mybir.dt
AF = mybir.ActivationFunctionType
ALU = mybir.AluOpType

B = int(os.environ.get("BASS_KERNEL_B", "65536"))
NCORES = int(os.environ.get("BASS_KERNEL_CORES", "8"))
BC = B // NCORES          # images per core
NT = min(1024, BC)        # images per tile
NTILES = BC // NT
NSUB = NT // 128          # fc2 image-subtiles per tile

# conv1 blocking: 12 blocks = 6 by (4 out rows each) x 2 bx (12 out cols each)
# patch per block: 8 rows x 16 cols = 128 K-rows (full)
# M layout: quad (yp, xq) * 32 + payload; payload = pc*4 + pr*2 + ch (24 used)


def _bf16(a):
    return np.asarray(a, dtype=np.float32).astype(np.float16)


# ---------------------------------------------------------------- host packing

def build_a(x):
    """x: [B, 784] -> [12, 128, B] bf16 im2col (block, patch-pixel, image)."""
    n = x.shape[0]
    xb = _bf16(x).reshape(n, 28, 28)
    out = np.empty((12, 128, n), dtype=np.float16)
    for by in range(6):
        for bx in range(2):
            b = by * 2 + bx
            for iy in range(8):
                src = xb[:, 4 * by + iy, 12 * bx:12 * bx + 16]   # [n,16]
                out[b, iy * 16:iy * 16 + 16, :] = src.T
    return out


def _m_decode(m):
    """conv1 M index -> (valid, dy, dx, ch): out pixel local coords."""
    quad, pay = m >> 5, m & 31
    if pay >= 24:
        return False, 0, 0, 0
    yp, xq = quad >> 1, quad & 1
    pc, r = pay >> 2, pay & 3
    pr, ch = r >> 1, r & 1
    return True, 2 * pr + yp, 2 * pc + xq, ch


def build_w1(w1):
    """w1: [2,1,5,5] -> [128, 12*128] bf16 (same lhsT for every block)."""
    W = np.zeros((128, 128), dtype=np.float32)
    w1 = np.asarray(w1, dtype=np.float32).reshape(2, 5, 5)
    for m in range(128):
        ok, dy, dx, ch = _m_decode(m)
        if not ok:
            continue
        for ky in range(5):
            for kx in range(5):
                W[(dy + ky) * 16 + (dx + kx), m] = w1[ch, ky, kx]
    return _bf16(W)


def build_w2(w2):
    """w2: [16,2,5,5] -> [128, 16*128] bf16.

    mm i = j*2 + bxi, j = (a, xh) = 2*a + xh.  K-tile (yh(j), bxi):
    K-row k = 32*slot + pc*4 + pr*2 + ch  (slot = chunk index 0..3)
      chunk by = slot + 2*yh;  pooled R = 2*by + pr, C = 6*bxi + pc.
    M col m = quad(yp2, xp2)*32 + oc*2 + bb:
      out pixel oy = 2*a + yp2, ox = 4*xh + 2*bb + xp2.
    """
    W = np.zeros((128, 16 * 128), dtype=np.float32)
    w2 = np.asarray(w2, dtype=np.float32)
    for a in range(4):
        yh = 0 if a < 2 else 1
        for xh in range(2):
            j = 2 * a + xh
            for bxi in range(2):
                col0 = (j * 2 + bxi) * 128
                for slot in range(4):
                    by = slot + 2 * yh
                    for pc in range(6):
                        for pr in range(2):
                            for ch in range(2):
                                k = 32 * slot + pc * 4 + pr * 2 + ch
                                R = 2 * by + pr
                                C = 6 * bxi + pc
                                for m in range(128):
                                    quad, pay = m >> 5, m & 31
                                    yp2, xp2 = quad >> 1, quad & 1
                                    oc, bb = pay >> 1, pay & 1
                                    oy = 2 * a + yp2
                                    ox = 4 * xh + 2 * bb + xp2
                                    ky, kx = R - oy, C - ox
                                    if 0 <= ky < 5 and 0 <= kx < 5:
                                        W[k, col0 + m] = w2[oc, ch, ky, kx]
    return _bf16(W)


def build_wfc1(fc1_w):
    """fc1_w: [64, 256] -> [128, 2*64] bf16 (2 K-tiles F2a/F2b)."""
    W = np.zeros((128, 2 * 64), dtype=np.float32)
    fc1_w = np.asarray(fc1_w, dtype=np.float32)
    for t in range(2):
        for p in range(128):
            jl, pay = p >> 5, p & 31
            j = 4 * t + jl
            a, xh = j >> 1, j & 1
            oc, bb = pay >> 1, pay & 1
            b_ = 2 * xh + bb
            flat = oc * 16 + a * 4 + b_
            W[p, t * 64:t * 64 + 64] = fc1_w[:, flat]
    return _bf16(W)


def head_constants(qnn_params, fc3_w, fc3_b, fc2_b):
    """Reduce the 2-qubit circuit tail + fc3 to z = c0 + sum Mk*cos(...)."""
    p = np.asarray(qnn_params, dtype=np.float64)

    def ry(t):
        c, s = np.cos(t), np.sin(t)
        return np.array([[c, -s], [s, c]])

    def kron_w0(U):
        return np.kron(U, np.eye(2))

    def kron_w1(U):
        return np.kron(np.eye(2), U)

    CN01 = np.zeros((4, 4)); CN01[0, 0] = CN01[1, 1] = 1; CN01[2, 3] = CN01[3, 2] = 1
    CN10 = np.zeros((4, 4))
    for q0 in range(2):
        for q1 in range(2):
            CN10[((q0 ^ q1) * 2 + q1), q0 * 2 + q1] = 1
    U = np.eye(4)
    U = kron_w0(ry(p[0])) @ U
    U = kron_w1(ry(p[1])) @ U
    U = CN01 @ U
    U = kron_w0(ry(p[2])) @ U
    U = kron_w1(ry(p[3])) @ U
    U = CN10 @ U
    U = kron_w0(ry(p[4])) @ U
    U = kron_w1(ry(p[5])) @ U
    U = CN01 @ U
    U = kron_w0(ry(p[6])) @ U
    U = kron_w1(ry(p[7])) @ U
    S = np.diag([1.0, -1.0, -1.0, 1.0])
    M = 0.25 * (U.T @ S @ U)
    w3 = float(np.asarray(fc3_w).reshape(()))
    b3 = float(np.asarray(fc3_b).reshape(()))
    c0 = float(np.trace(M)) * w3 + b3
    k = {
        "A": 2 * M[0, 3] * w3,   # cos(2x0+2x1)
        "B": 2 * M[1, 2] * w3,   # cos(2x0-2x1)
        "C": 2 * M[0, 2] * w3,   # cos(2x0+2ang)
        "D": 2 * M[1, 3] * w3,   # cos(2x0-2ang)
        "E": 2 * M[0, 1] * w3,   # cos(2x1+2ang)
        "F": 2 * M[2, 3] * w3,   # cos(2x1-2ang)
    }
    return c0, k, float(fc2_b[0]), float(fc2_b[1])


# ---------------------------------------------------------------- bass program

def build_program(weights):
    nc = bacc.Bacc(None, target_bir_lowering=False, debug=False)
    a_d = nc.declare_dram_parameter("a_c1", [12, 128, BC], dt.float16, isOutput=False)
    w1_d = nc.declare_dram_parameter("w1", [128, 128], dt.float16, isOutput=False)
    w2_d = nc.declare_dram_parameter("w2", [128, 16 * 128], dt.float16, isOutput=False)
    wf1_d = nc.declare_dram_parameter("wf1", [128, 2 * 64], dt.float16, isOutput=False)
    wf2_d = nc.declare_dram_parameter("wf2", [64, 2], dt.float16, isOutput=False)
    cst_d = nc.declare_dram_parameter("cst", [128, 16], dt.float32, isOutput=False)
    y_d = nc.declare_dram_parameter("y", [2, BC], dt.float32, isOutput=True)

    c0, K, b20, b21 = weights["head"]
    pi = float(np.pi)

    with tile.TileContext(nc) as tc:
        with tc.tile_pool(name="cw", bufs=1) as cw, \
             tc.tile_pool(name="sx", bufs=2) as sx, \
             tc.tile_pool(name="sc", bufs=3) as sc, \
             tc.tile_pool(name="sm", bufs=2) as sm, \
             tc.tile_pool(name="se", bufs=2) as se, \
             tc.tile_pool(name="sf", bufs=2) as sf, \
             tc.tile_pool(name="hd", bufs=1) as hd, \
             tc.tile_pool(name="p1", bufs=2, space="PSUM") as p1:

            W1 = cw.tile([128, 128], dt.float16)
            nc.sync.dma_start(out=W1[:], in_=w1_d[:])
            W2 = cw.tile([128, 16 * 128], dt.float16)
            nc.sync.dma_start(out=W2[:], in_=w2_d[:])
            WF1 = cw.tile([128, 2 * 64], dt.float16)
            nc.sync.dma_start(out=WF1[:], in_=wf1_d[:])
            WF2 = cw.tile([64, 2], dt.float16)
            nc.sync.dma_start(out=WF2[:], in_=wf2_d[:])
            CST = cw.tile([128, 16], dt.float32)
            nc.sync.dma_start(out=CST[:], in_=cst_d[:])
            Hall = cw.tile([128, NTILES * 2 * NSUB], dt.float32)

            # engine rotation: drains on DVE/Act (GpSimd cannot read PSUM),
            # SBUF-only pool stages on DVE/GpSimd
            drain_eng = [nc.scalar, nc.scalar, nc.scalar, nc.vector]
            tt_eng = [nc.vector, nc.vector]

            for it in range(NTILES):
                n0 = it * NT
                xc = sx.tile([128, 12 * NT], dt.float16)
                nc.sync.dma_start(
                    out=xc[:].rearrange("p (b n) -> p b n", b=12),
                    in_=a_d[:, :, n0:n0 + NT].transpose([1, 0, 2]))

                # T K-tiles for conv2: [yh] each [128, 2*NT], free = (bx, n)
                T = [se.tile([128, 2 * NT], dt.float16, tag=f"T{yh}",
                             name=f"T{yh}") for yh in range(2)]

                di = 0
                ti = 0
                # conv1: block pairs (by, bx=0/1) share weights -> one matmul
                # of N=2*NT into a 4-bank psum tile
                for by in range(6):
                    ps = p1.tile([128, 2 * NT], dt.float32, tag="ps")
                    for q in range(2 * NT // 512):
                        c0_ = 2 * by * NT + q * 512
                        nc.tensor.matmul(out=ps[:, q * 512:(q + 1) * 512],
                                         lhsT=W1[:, 0:128],
                                         rhs=xc[:, c0_:c0_ + 512],
                                         start=True, stop=True)
                    # drain + conv1 bias (per-partition AP scalar)
                    C = sc.tile([128, 2 * NT], dt.float16, tag="C")
                    eng = drain_eng[di % 2]; di += 1
                    if eng is nc.scalar:
                        nc.scalar.activation(out=C[:], in_=ps[:], func=AF.Relu,
                                             bias=CST[:, 0:1])
                    else:
                        eng.tensor_scalar(out=C[:], in0=ps[:],
                                          scalar1=CST[:, 0:1], scalar2=0.0,
                                          op0=ALU.add, op1=ALU.max)
                    # stage1: max over yp; upper half copied to base-0 first
                    # (two-input SBUF ops require equal base partitions)
                    S1 = sm.tile([64, 2 * NT], dt.float16, tag="S1")
                    nc.sync.dma_start(out=S1[:], in_=C[64:128, :])
                    M1 = sm.tile([64, 2 * NT], dt.float16, tag="M1")
                    eng = tt_eng[ti % 2]; ti += 1
                    eng.tensor_tensor(out=M1[:], in0=C[0:64, :], in1=S1[:],
                                      op=ALU.max)
                    S2 = sm.tile([32, 2 * NT], dt.float16, tag="S2")
                    nc.sync.dma_start(out=S2[:], in_=M1[32:64, :])
                    # stage2: max over xq + relu -> chunk row of T (both blocks)
                    yh = 0 if by < 4 else 1
                    slot = by - 2 * yh
                    eng = tt_eng[ti % 2]; ti += 1
                    eng.tensor_tensor(
                        out=T[yh][32 * slot:32 * slot + 32, :],
                        in0=M1[0:32, :], in1=S2[:], op=ALU.max)

                # chunks by=2,3 (T0 slots 2,3) also open T1 as slots 0,1
                nc.sync.dma_start(out=T[1][0:64, :], in_=T[0][64:128, :])

                # conv2: 8 out-tiles x 2 accumulated matmuls; pairs (j, j+1)
                # share one 4-bank psum tile and batched drain/stage1
                F2 = [sf.tile([128, NT], dt.float16, tag=f"F2{t}", name=f"F2{t}")
                      for t in range(2)]
                for a_ in range(4):
                    yh = 0 if a_ < 2 else 1
                    ps2 = p1.tile([128, 2 * NT], dt.float32, tag="ps")
                    for xh in range(2):
                        j = 2 * a_ + xh
                        for bxi in range(2):
                            for q in range(NT // 512):
                                nc.tensor.matmul(
                                    out=ps2[:, xh * NT + q * 512:xh * NT + (q + 1) * 512],
                                    lhsT=W2[:, (j * 2 + bxi) * 128:(j * 2 + bxi + 1) * 128],
                                    rhs=T[yh][:, bxi * NT + q * 512:bxi * NT + (q + 1) * 512],
                                    start=(bxi == 0), stop=(bxi == 1))
                    D = sc.tile([128, 2 * NT], dt.float16, tag="D")
                    eng = drain_eng[di % 2]; di += 1
                    if eng is nc.scalar:
                        nc.scalar.activation(out=D[:], in_=ps2[:], func=AF.Relu,
                                             bias=CST[:, 1:2])
                    else:
                        eng.tensor_scalar(out=D[:], in0=ps2[:],
                                          scalar1=CST[:, 1:2], scalar2=0.0,
                                          op0=ALU.add, op1=ALU.max)
                    S1b = sm.tile([64, 2 * NT], dt.float16, tag="S1b")
                    nc.sync.dma_start(out=S1b[:], in_=D[64:128, :])
                    M2 = sm.tile([64, 2 * NT], dt.float16, tag="M2")
                    eng = tt_eng[ti % 2]; ti += 1
                    eng.tensor_tensor(out=M2[:], in0=D[0:64, :], in1=S1b[:],
                                      op=ALU.max)
                    S2b = sm.tile([32, 2 * NT], dt.float16, tag="S2b")
                    nc.sync.dma_start(out=S2b[:], in_=M2[32:64, :])
                    for xh in range(2):
                        j = 2 * a_ + xh
                        ft, jl = j // 4, j % 4
                        eng = tt_eng[ti % 2]; ti += 1
                        eng.tensor_tensor(
                            out=F2[ft][32 * jl:32 * jl + 32, :],
                            in0=M2[0:32, xh * NT:(xh + 1) * NT],
                            in1=S2b[:, xh * NT:(xh + 1) * NT], op=ALU.max)

                # fc1 (K=256 over 2 tiles) -> relu -> F1 bf16
                psf = p1.tile([64, 2 * NT], dt.float32, tag="ps")
                for t_ in range(2):
                    for q in range(NT // 512):
                        nc.tensor.matmul(out=psf[:, q * 512:(q + 1) * 512],
                                         lhsT=WF1[:, 64 * t_:64 * (t_ + 1)],
                                         rhs=F2[t_][:, q * 512:(q + 1) * 512],
                                         start=(t_ == 0), stop=(t_ == 1))
                F1 = sf.tile([64, NT], dt.float16, tag="F1")
                nc.scalar.activation(out=F1[:], in_=psf[:, 0:NT], func=AF.Relu,
                                     bias=CST[0:64, 2:3])

                # fc2 img-major: NSUB matmuls N=2 -> psum [128, 2*NSUB]
                psg = p1.tile([128, 2 * NT], dt.float32, tag="ps")
                for s in range(NSUB):
                    nc.tensor.matmul(out=psg[:, 2 * s:2 * s + 2],
                                     lhsT=F1[:, 128 * s:128 * (s + 1)],
                                     rhs=WF2[:], start=True, stop=True)
                nc.vector.tensor_copy(out=Hall[:, it * 2 * NSUB:(it + 1) * 2 * NSUB],
                                      in_=psg[:, 0:2 * NSUB])

            # ---- head once: Hall [128, (t, s, c)] -> Yall [128, (t, s)]
            NC_ = NTILES * NSUB       # head column count
            Hv = Hall[:].rearrange("p (u c) -> p u c", c=2)
            x0 = Hv[:, :, 0]
            x1 = Hv[:, :, 1]
            t0 = hd.tile([128, NC_], dt.float32)
            nc.vector.tensor_scalar(out=t0[:], in0=x0, scalar1=-1.0,
                                    scalar2=pi - b20, op0=ALU.mult, op1=ALU.add)
            t1 = hd.tile([128, NC_], dt.float32)
            nc.vector.tensor_scalar(out=t1[:], in0=x1, scalar1=-1.0,
                                    scalar2=pi - b21, op0=ALU.mult, op1=ALU.add)
            ang = hd.tile([128, NC_], dt.float32)
            nc.vector.tensor_tensor(out=ang[:], in0=t0[:], in1=t1[:], op=ALU.mult)

            qpi = pi / 4
            hb = {"A": b20 + b21 + qpi, "B": b20 - b21 + qpi,
                  "C": b20 + qpi, "D": b20 + qpi,
                  "E": b21 + qpi, "F": b21 + qpi}
            AR = hd.tile([128, 6 * NC_], dt.float32)
            plan = (("A", x0, x1, ALU.add), ("B", x0, x1, ALU.subtract),
                    ("C", x0, ang[:], ALU.add), ("D", x0, ang[:], ALU.subtract),
                    ("E", x1, ang[:], ALU.add), ("F", x1, ang[:], ALU.subtract))
            for i, (nm, a0, a1, op) in enumerate(plan):
                eng = tt_eng[i % 2]
                eng.scalar_tensor_tensor(
                    out=AR[:, NC_ * i:NC_ * (i + 1)], in0=a0, scalar=hb[nm],
                    in1=a1, op0=ALU.add, op1=op)
            # range reduce: h = AR - pi*round(AR/pi); sin(2h) = sin(2AR mod 2pi)
            tq = hd.tile([128, 6 * NC_], dt.float32)
            nc.vector.tensor_scalar(out=tq[:], in0=AR[:], scalar1=float(1 / pi),
                                    scalar2=None, op0=ALU.mult)
            ti_ = hd.tile([128, 6 * NC_], dt.int32)
            nc.vector.tensor_copy(out=ti_[:], in_=tq[:])
            tf_ = hd.tile([128, 6 * NC_], dt.float32)
            nc.vector.tensor_copy(out=tf_[:], in_=ti_[:])
            hh = hd.tile([128, 6 * NC_], dt.float32)
            nc.vector.scalar_tensor_tensor(out=hh[:], in0=tf_[:], scalar=-pi,
                                           in1=AR[:], op0=ALU.mult, op1=ALU.add)
            SN = hd.tile([128, 6 * NC_], dt.float32)
            nc.scalar.activation(out=SN[:], in_=hh[:], func=AF.Sin, scale=2.0)
            cosv = {nm: SN[:, NC_ * i:NC_ * (i + 1)]
                    for i, nm in enumerate("ABCDEF")}

            acc = hd.tile([128, NC_], dt.float32, tag="acc0")
            nc.vector.tensor_scalar(out=acc[:], in0=cosv["A"], scalar1=K["A"],
                                    scalar2=c0, op0=ALU.mult, op1=ALU.add)
            for i, nm in enumerate("BCDE"):
                acc2 = hd.tile([128, NC_], dt.float32, tag=f"acc{i+1}")
                eng = tt_eng[i % 2]
                eng.scalar_tensor_tensor(out=acc2[:], in0=cosv[nm][:],
                                         scalar=K[nm], in1=acc[:],
                                         op0=ALU.mult, op1=ALU.add)
                acc = acc2
            Yall = hd.tile([128, NC_], dt.float32, tag="Yall")
            nc.vector.scalar_tensor_tensor(out=Yall[:], in0=cosv["F"],
                                           scalar=K["F"], in1=acc[:],
                                           op0=ALU.mult, op1=ALU.add)

            # ---- final: out0=-ln(1+e^{1-2y}), out1=-ln(1+e^{2y-1})
            V = hd.tile([128, NC_], dt.float32)
            nc.scalar.activation(out=V[:], in_=Yall[:], func=AF.Exp,
                                 bias=CST[:, 8:9], scale=-2.0)      # e^{1-2y}
            Wr = hd.tile([128, NC_], dt.float32)
            nc.vector.reciprocal(out=Wr[:], in_=V[:])               # e^{2y-1}
            L0 = hd.tile([128, NC_], dt.float32)
            nc.scalar.activation(out=L0[:], in_=V[:], func=AF.Ln,
                                 bias=CST[:, 8:9], scale=1.0)       # ln(1+v)
            L1 = hd.tile([128, NC_], dt.float32)
            nc.scalar.activation(out=L1[:], in_=Wr[:], func=AF.Ln,
                                 bias=CST[:, 8:9], scale=1.0)
            O = hd.tile([128, 2 * NC_], dt.float32)
            Ov = O[:].rearrange("p (c u) -> p c u", c=2)
            nc.vector.tensor_scalar(out=Ov[:, 0, :], in0=L0[:], scalar1=-1.0,
                                    scalar2=None, op0=ALU.mult)
            nc.vector.tensor_scalar(out=Ov[:, 1, :], in0=L1[:], scalar1=-1.0,
                                    scalar2=None, op0=ALU.mult)
            # y layout [2, BC]: dst[c, it*NT + s*128 + p] <- O[p, (c, it, s)]
            for c in range(2):
                nc.sync.dma_start(
                    out=y_d[c, :].rearrange("(t s p) -> p t s", p=128, s=NSUB),
                    in_=Ov[:, c, :].rearrange("p (t s) -> p t s", s=NSUB))

    nc.compile()
    return nc


def kernel(x, conv1_w, conv1_b, conv2_w, conv2_b, fc1_w, fc1_b,
           fc2_w, fc2_b, fc3_w, fc3_b, qnn_params):
    x = np.asarray(x, dtype=np.float32).reshape(B, 784)
    a = build_a(x)
    W1 = build_w1(conv1_w)
    W2 = build_w2(conv2_w)
    WF1 = build_wfc1(fc1_w)
    WF2 = _bf16(np.asarray(fc2_w, np.float32).T)  # [64, 2]
    c0, K, b20, b21 = head_constants(qnn_params, fc3_w, fc3_b,
                                     np.asarray(fc2_b, np.float32))
    cst = np.zeros((128, 16), dtype=np.float32)
    b1 = np.asarray(conv1_b, np.float32)
    b2 = np.asarray(conv2_b, np.float32)
    for p in range(128):
        pay = p & 31
        cst[p, 0] = b1[pay & 1] if pay < 24 else 0.0   # conv1 bias (ch = p&1)
        cst[p, 1] = b2[pay >> 1]                        # conv2 bias (oc)
    cst[0:64, 2] = np.asarray(fc1_b, np.float32)
    cst[:, 8] = 1.0

    weights = {"head": (c0, K, b20, b21)}
    nc = build_program(weights)

    in_maps = []
    for c in range(NCORES):
        sl = slice(c * BC, (c + 1) * BC)
        in_maps.append({
            "a_c1": np.ascontiguousarray(a[:, :, sl]),
            "w1": W1, "w2": W2, "wf1": WF1, "wf2": WF2, "cst": cst,
        })
    res = run_bass_kernel_spmd(nc, in_maps, list(range(NCORES)),
                               trace=bool(int(os.environ.get("BASS_TRACE_KERNEL", "0"))))
    if res.exec_time_ns is not None:
        print(f"HW exec time: {res.exec_time_ns} ns")
    global LAST_RESULTS
    LAST_RESULTS = res.results
    out = np.empty((B, 2), dtype=np.float32)
    for c in range(NCORES):
        out[c * BC:(c + 1) * BC] = res.results[c]["y"].T
    return out


# revision 28
# speedup vs baseline: 3.2716x; 1.0062x over previous
"""Trainium2 Bass kernel for nn_BinaryClassifier (CNN + 2-qubit circuit head).

Data-parallel over 8 cores (65536 -> 8192/core), NT=1024 images per tile.

Structure per tile:
  conv1: 12 blocks (4y x 12x out, patch 8x16 = K 128 full), 1 matmul each.
         Bias folded into the PSUM drain (tensor_scalar w/ AP bias).
  pool1: drain -> stage1 max(yp halves, cross-partition-offset TT)
         -> stage2 max+relu (STT) writing 32-row chunks into conv2 K-tiles.
  conv2: 8 out-tiles x 2 accumulated matmuls over [128,N] K-tiles
         (zero weight rows where a row is outside the 6x8 patch).
  pool2: same pattern -> fc1 rhs tiles F2a/F2b.
  fc1:   2 matmuls + Act relu/bias drain.  fc2: 8 small matmuls.
  head:  quantum circuit reduced to 6 cosines, computed once after the loop.
"""
import os, sys
sys.path.insert(0, "/opt/trn_rl_repo")
import numpy as np
import ml_dtypes

from concourse import bass, tile, bacc
from concourse import mybir
from concourse.bass_utils import run_bass_kernel_spmd

dt = mybir.dt
AF = mybir.ActivationFunctionType
ALU = mybir.AluOpType

B = int(os.environ.get("BASS_KERNEL_B", "65536"))
NCORES = int(os.environ.get("BASS_KERNEL_CORES", "8"))
BC = B // NCORES          # images per core
NT = min(1024, BC)        # images per tile
NTILES = BC // NT
NSUB = NT // 128          # fc2 image-subtiles per tile

# conv1 blocking: 12 blocks = 6 by (4 out rows each) x 2 bx (12 out cols each)
# patch per block: 8 rows x 16 cols = 128 K-rows (full)
# M layout: quad (yp, xq) * 32 + payload; payload = pc*4 + pr*2 + ch (24 used)


def _bf16(a):
    return np.asarray(a, dtype=np.float32).astype(np.float16)


# ---------------------------------------------------------------- host packing

def build_a(x):
    """x: [B, 784] -> [12, 128, B] bf16 im2col (block, patch-pixel, image)."""
    n = x.shape[0]
    xb = _bf16(x).reshape(n, 28, 28)
    out = np.empty((12, 128, n), dtype=np.float16)
    for by in range(6):
        for bx in range(2):
            b = by * 2 + bx
            for iy in range(8):
                src = xb[:, 4 * by + iy, 12 * bx:12 * bx + 16]   # [n,16]
                out[b, iy * 16:iy * 16 + 16, :] = src.T
    return out


def _m_decode(m):
    """conv1 M index -> (valid, dy, dx, ch): out pixel local coords."""
    quad, pay = m >> 5, m & 31
    if pay >= 24:
        return False, 0, 0, 0
    yp, xq = quad >> 1, quad & 1
    pc, r = pay >> 2, pay & 3
    pr, ch = r >> 1, r & 1
    return True, 2 * pr + yp, 2 * pc + xq, ch


def build_w1(w1):
    """w1: [2,1,5,5] -> [128, 12*128] bf16 (same lhsT for every block)."""
    W = np.zeros((128, 128), dtype=np.float32)
    w1 = np.asarray(w1, dtype=np.float32).reshape(2, 5, 5)
    for m in range(128):
        ok, dy, dx, ch = _m_decode(m)
        if not ok:
            continue
        for ky in range(5):
            for kx in range(5):
                W[(dy + ky) * 16 + (dx + kx), m] = w1[ch, ky, kx]
    return _bf16(W)


def build_w2(w2):
    """w2: [16,2,5,5] -> [128, 16*128] bf16.

    mm i = j*2 + bxi, j = (a, xh) = 2*a + xh.  K-tile (yh(j), bxi):
    K-row k = 32*slot + pc*4 + pr*2 + ch  (slot = chunk index 0..3)
      chunk by = slot + 2*yh;  pooled R = 2*by + pr, C = 6*bxi + pc.
    M col m = quad(yp2, xp2)*32 + oc*2 + bb:
      out pixel oy = 2*a + yp2, ox = 4*xh + 2*bb + xp2.
    """
    W = np.zeros((128, 16 * 128), dtype=np.float32)
    w2 = np.asarray(w2, dtype=np.float32)
    for a in range(4):
        yh = 0 if a < 2 else 1
        for xh in range(2):
            j = 2 * a + xh
            for bxi in range(2):
                col0 = (j * 2 + bxi) * 128
                for slot in range(4):
                    by = slot + 2 * yh
                    for pc in range(6):
                        for pr in range(2):
                            for ch in range(2):
                                k = 32 * slot + pc * 4 + pr * 2 + ch
                                R = 2 * by + pr
                                C = 6 * bxi + pc
                                for m in range(128):
                                    quad, pay = m >> 5, m & 31
                                    yp2, xp2 = quad >> 1, quad & 1
                                    oc, bb = pay >> 1, pay & 1
                                    oy = 2 * a + yp2
                                    ox = 4 * xh + 2 * bb + xp2
                                    ky, kx = R - oy, C - ox
                                    if 0 <= ky < 5 and 0 <= kx < 5:
                                        W[k, col0 + m] = w2[oc, ch, ky, kx]
    return _bf16(W)


def build_wfc1(fc1_w):
    """fc1_w: [64, 256] -> [128, 2*64] bf16 (2 K-tiles F2a/F2b)."""
    W = np.zeros((128, 2 * 64), dtype=np.float32)
    fc1_w = np.asarray(fc1_w, dtype=np.float32)
    for t in range(2):
        for p in range(128):
            jl, pay = p >> 5, p & 31
            j = 4 * t + jl
            a, xh = j >> 1, j & 1
            oc, bb = pay >> 1, pay & 1
            b_ = 2 * xh + bb
            flat = oc * 16 + a * 4 + b_
            W[p, t * 64:t * 64 + 64] = fc1_w[:, flat]
    return _bf16(W)


def head_constants(qnn_params, fc3_w, fc3_b, fc2_b):
    """Reduce the 2-qubit circuit tail + fc3 to z = c0 + sum Mk*cos(...)."""
    p = np.asarray(qnn_params, dtype=np.float64)

    def ry(t):
        c, s = np.cos(t), np.sin(t)
        return np.array([[c, -s], [s, c]])

    def kron_w0(U):
        return np.kron(U, np.eye(2))

    def kron_w1(U):
        return np.kron(np.eye(2), U)

    CN01 = np.zeros((4, 4)); CN01[0, 0] = CN01[1, 1] = 1; CN01[2, 3] = CN01[3, 2] = 1
    CN10 = np.zeros((4, 4))
    for q0 in range(2):
        for q1 in range(2):
            CN10[((q0 ^ q1) * 2 + q1), q0 * 2 + q1] = 1
    U = np.eye(4)
    U = kron_w0(ry(p[0])) @ U
    U = kron_w1(ry(p[1])) @ U
    U = CN01 @ U
    U = kron_w0(ry(p[2])) @ U
    U = kron_w1(ry(p[3])) @ U
    U = CN10 @ U
    U = kron_w0(ry(p[4])) @ U
    U = kron_w1(ry(p[5])) @ U
    U = CN01 @ U
    U = kron_w0(ry(p[6])) @ U
    U = kron_w1(ry(p[7])) @ U
    S = np.diag([1.0, -1.0, -1.0, 1.0])
    M = 0.25 * (U.T @ S @ U)
    w3 = float(np.asarray(fc3_w).reshape(()))
    b3 = float(np.asarray(fc3_b).reshape(()))
    c0 = float(np.trace(M)) * w3 + b3
    k = {
        "A": 2 * M[0, 3] * w3,   # cos(2x0+2x1)
        "B": 2 * M[1, 2] * w3,   # cos(2x0-2x1)
        "C": 2 * M[0, 2] * w3,   # cos(2x0+2ang)
        "D": 2 * M[1, 3] * w3,   # cos(2x0-2ang)
        "E": 2 * M[0, 1] * w3,   # cos(2x1+2ang)
        "F": 2 * M[2, 3] * w3,   # cos(2x1-2ang)
    }
    return c0, k, float(fc2_b[0]), float(fc2_b[1])


# ---------------------------------------------------------------- bass program

def build_program(weights):
    nc = bacc.Bacc(None, target_bir_lowering=False, debug=False)
    a_d = nc.declare_dram_parameter("a_c1", [12, 128, BC], dt.float16, isOutput=False)
    w1_d = nc.declare_dram_parameter("w1", [128, 128], dt.float16, isOutput=False)
    w2_d = nc.declare_dram_parameter("w2", [128, 16 * 128], dt.float16, isOutput=False)
    wf1_d = nc.declare_dram_parameter("wf1", [128, 2 * 64], dt.float16, isOutput=False)
    wf2_d = nc.declare_dram_parameter("wf2", [64, 2], dt.float16, isOutput=False)
    cst_d = nc.declare_dram_parameter("cst", [128, 16], dt.float32, isOutput=False)
    y_d = nc.declare_dram_parameter("y", [2, BC], dt.float32, isOutput=True)

    c0, K, b20, b21 = weights["head"]
    pi = float(np.pi)

    with tile.TileContext(nc) as tc:
        with tc.tile_pool(name="cw", bufs=1) as cw, \
             tc.tile_pool(name="sx", bufs=2) as sx, \
             tc.tile_pool(name="sc", bufs=3) as sc, \
             tc.tile_pool(name="sm", bufs=2) as sm, \
             tc.tile_pool(name="se", bufs=2) as se, \
             tc.tile_pool(name="sf", bufs=2) as sf, \
             tc.tile_pool(name="hd", bufs=1) as hd, \
             tc.tile_pool(name="p1", bufs=4, space="PSUM") as p1:

            W1 = cw.tile([128, 128], dt.float16)
            nc.sync.dma_start(out=W1[:], in_=w1_d[:])
            W2 = cw.tile([128, 16 * 128], dt.float16)
            nc.sync.dma_start(out=W2[:], in_=w2_d[:])
            WF1 = cw.tile([128, 2 * 64], dt.float16)
            nc.sync.dma_start(out=WF1[:], in_=wf1_d[:])
            WF2 = cw.tile([64, 2], dt.float16)
            nc.sync.dma_start(out=WF2[:], in_=wf2_d[:])
            CST = cw.tile([128, 16], dt.float32)
            nc.sync.dma_start(out=CST[:], in_=cst_d[:])
            Hall = cw.tile([128, NTILES * 2 * NSUB], dt.float32)

            # engine rotation: drains on DVE/Act (GpSimd cannot read PSUM),
            # SBUF-only pool stages on DVE/GpSimd
            drain_eng = [nc.scalar]
            tt_eng = [nc.vector, nc.vector]

            for it in range(NTILES):
                n0 = it * NT
                xc = sx.tile([128, 12 * NT], dt.float16)
                nc.sync.dma_start(
                    out=xc[:].rearrange("p (b n) -> p b n", b=12),
                    in_=a_d[:, :, n0:n0 + NT].transpose([1, 0, 2]))

                # T K-tiles for conv2: [yh] each [128, 2*NT], free = (bx, n)
                T = [se.tile([128, 2 * NT], dt.float16, tag=f"T{yh}",
                             name=f"T{yh}") for yh in range(2)]

                di = 0
                ti = 0
                # conv1: block pairs (by, bx=0/1) share weights -> one matmul
                # of N=2*NT into a 4-bank psum tile
                for by in range(6):
                    psu = [p1.tile([128, NT], dt.float32, tag="ps", name=f"ps{by}{h}")
                           for h in range(2)]
                    for q in range(2 * NT // 512):
                        c0_ = 2 * by * NT + q * 512
                        ph_, qh = q // (NT // 512), q % (NT // 512)
                        nc.tensor.matmul(out=psu[ph_][:, qh * 512:(qh + 1) * 512],
                                         lhsT=W1[:, 0:128],
                                         rhs=xc[:, c0_:c0_ + 512],
                                         start=True, stop=True)
                    # drain + conv1 bias + relu (per-partition AP scalar)
                    C = sc.tile([128, 2 * NT], dt.float16, tag="C")
                    for h in range(2):
                        eng = drain_eng[di % len(drain_eng)]; di += 1
                        if eng is nc.scalar:
                            nc.scalar.activation(out=C[:, h * NT:(h + 1) * NT],
                                                 in_=psu[h][:], func=AF.Relu,
                                                 bias=CST[:, 0:1])
                        else:
                            eng.tensor_scalar(out=C[:, h * NT:(h + 1) * NT],
                                              in0=psu[h][:],
                                              scalar1=CST[:, 0:1], scalar2=0.0,
                                              op0=ALU.add, op1=ALU.max)
                    # stage1: max over yp; upper half copied to base-0 first
                    # (two-input SBUF ops require equal base partitions)
                    S1 = sm.tile([64, 2 * NT], dt.float16, tag="S1")
                    nc.sync.dma_start(out=S1[:], in_=C[64:128, :])
                    M1 = sm.tile([64, 2 * NT], dt.float16, tag="M1")
                    eng = tt_eng[ti % 2]; ti += 1
                    eng.tensor_tensor(out=M1[:], in0=C[0:64, :], in1=S1[:],
                                      op=ALU.max)
                    S2 = sm.tile([32, 2 * NT], dt.float16, tag="S2")
                    nc.sync.dma_start(out=S2[:], in_=M1[32:64, :])
                    # stage2: max over xq + relu -> chunk row of T (both blocks)
                    yh = 0 if by < 4 else 1
                    slot = by - 2 * yh
                    eng = tt_eng[ti % 2]; ti += 1
                    eng.tensor_tensor(
                        out=T[yh][32 * slot:32 * slot + 32, :],
                        in0=M1[0:32, :], in1=S2[:], op=ALU.max)

                # chunks by=2,3 (T0 slots 2,3) also open T1 as slots 0,1
                nc.sync.dma_start(out=T[1][0:64, :], in_=T[0][64:128, :])

                # conv2: 8 out-tiles x 2 accumulated matmuls; pairs (j, j+1)
                # share one 4-bank psum tile and batched drain/stage1
                F2 = [sf.tile([128, NT], dt.float16, tag=f"F2{t}", name=f"F2{t}")
                      for t in range(2)]
                for a_ in range(4):
                    yh = 0 if a_ < 2 else 1
                    ps2 = [p1.tile([128, NT], dt.float32, tag="ps", name=f"p2{a_}{h}")
                           for h in range(2)]
                    for xh in range(2):
                        j = 2 * a_ + xh
                        for bxi in range(2):
                            for q in range(NT // 512):
                                nc.tensor.matmul(
                                    out=ps2[xh][:, q * 512:(q + 1) * 512],
                                    lhsT=W2[:, (j * 2 + bxi) * 128:(j * 2 + bxi + 1) * 128],
                                    rhs=T[yh][:, bxi * NT + q * 512:bxi * NT + (q + 1) * 512],
                                    start=(bxi == 0), stop=(bxi == 1))
                    D = sc.tile([128, 2 * NT], dt.float16, tag="D")
                    for h in range(2):
                        eng = drain_eng[di % len(drain_eng)]; di += 1
                        if eng is nc.scalar:
                            nc.scalar.activation(out=D[:, h * NT:(h + 1) * NT],
                                                 in_=ps2[h][:], func=AF.Relu,
                                                 bias=CST[:, 1:2])
                        else:
                            eng.tensor_scalar(out=D[:, h * NT:(h + 1) * NT],
                                              in0=ps2[h][:],
                                              scalar1=CST[:, 1:2], scalar2=0.0,
                                              op0=ALU.add, op1=ALU.max)
                    S1b = sm.tile([64, 2 * NT], dt.float16, tag="S1b")
                    nc.sync.dma_start(out=S1b[:], in_=D[64:128, :])
                    M2 = sm.tile([64, 2 * NT], dt.float16, tag="M2")
                    eng = tt_eng[ti % 2]; ti += 1
                    eng.tensor_tensor(out=M2[:], in0=D[0:64, :], in1=S1b[:],
                                      op=ALU.max)
                    S2b = sm.tile([32, 2 * NT], dt.float16, tag="S2b")
                    nc.sync.dma_start(out=S2b[:], in_=M2[32:64, :])
                    for xh in range(2):
                        j = 2 * a_ + xh
                        ft, jl = j // 4, j % 4
                        eng = tt_eng[ti % 2]; ti += 1
                        eng.tensor_tensor(
                            out=F2[ft][32 * jl:32 * jl + 32, :],
                            in0=M2[0:32, xh * NT:(xh + 1) * NT],
                            in1=S2b[:, xh * NT:(xh + 1) * NT], op=ALU.max)

                # fc1 (K=256 over 2 tiles) -> relu -> F1 bf16
                psf = p1.tile([64, NT], dt.float32, tag="ps")
                for t_ in range(2):
                    for q in range(NT // 512):
                        nc.tensor.matmul(out=psf[:, q * 512:(q + 1) * 512],
                                         lhsT=WF1[:, 64 * t_:64 * (t_ + 1)],
                                         rhs=F2[t_][:, q * 512:(q + 1) * 512],
                                         start=(t_ == 0), stop=(t_ == 1))
                F1 = sf.tile([64, NT], dt.float16, tag="F1")
                nc.scalar.activation(out=F1[:], in_=psf[:, 0:NT], func=AF.Relu,
                                     bias=CST[0:64, 2:3])

                # fc2 img-major: NSUB matmuls N=2 -> psum [128, 2*NSUB]
                psg = p1.tile([128, NT], dt.float32, tag="ps")
                for s in range(NSUB):
                    nc.tensor.matmul(out=psg[:, 2 * s:2 * s + 2],
                                     lhsT=F1[:, 128 * s:128 * (s + 1)],
                                     rhs=WF2[:], start=True, stop=True)
                nc.vector.tensor_copy(out=Hall[:, it * 2 * NSUB:(it + 1) * 2 * NSUB],
                                      in_=psg[:, 0:2 * NSUB])

            # ---- head once: Hall [128, (t, s, c)] -> Yall [128, (t, s)]
            NC_ = NTILES * NSUB       # head column count
            Hv = Hall[:].rearrange("p (u c) -> p u c", c=2)
            x0 = Hv[:, :, 0]
            x1 = Hv[:, :, 1]
            t0 = hd.tile([128, NC_], dt.float32)
            nc.vector.tensor_scalar(out=t0[:], in0=x0, scalar1=-1.0,
                                    scalar2=pi - b20, op0=ALU.mult, op1=ALU.add)
            t1 = hd.tile([128, NC_], dt.float32)
            nc.vector.tensor_scalar(out=t1[:], in0=x1, scalar1=-1.0,
                                    scalar2=pi - b21, op0=ALU.mult, op1=ALU.add)
            ang = hd.tile([128, NC_], dt.float32)
            nc.vector.tensor_tensor(out=ang[:], in0=t0[:], in1=t1[:], op=ALU.mult)

            qpi = pi / 4
            hb = {"A": b20 + b21 + qpi, "B": b20 - b21 + qpi,
                  "C": b20 + qpi, "D": b20 + qpi,
                  "E": b21 + qpi, "F": b21 + qpi}
            AR = hd.tile([128, 6 * NC_], dt.float32)
            plan = (("A", x0, x1, ALU.add), ("B", x0, x1, ALU.subtract),
                    ("C", x0, ang[:], ALU.add), ("D", x0, ang[:], ALU.subtract),
                    ("E", x1, ang[:], ALU.add), ("F", x1, ang[:], ALU.subtract))
            for i, (nm, a0, a1, op) in enumerate(plan):
                eng = tt_eng[i % 2]
                eng.scalar_tensor_tensor(
                    out=AR[:, NC_ * i:NC_ * (i + 1)], in0=a0, scalar=hb[nm],
                    in1=a1, op0=ALU.add, op1=op)
            # range reduce: h = AR - pi*round(AR/pi); sin(2h) = sin(2AR mod 2pi)
            tq = hd.tile([128, 6 * NC_], dt.float32)
            nc.vector.tensor_scalar(out=tq[:], in0=AR[:], scalar1=float(1 / pi),
                                    scalar2=None, op0=ALU.mult)
            ti_ = hd.tile([128, 6 * NC_], dt.int32)
            nc.vector.tensor_copy(out=ti_[:], in_=tq[:])
            tf_ = hd.tile([128, 6 * NC_], dt.float32)
            nc.vector.tensor_copy(out=tf_[:], in_=ti_[:])
            hh = hd.tile([128, 6 * NC_], dt.float32)
            nc.vector.scalar_tensor_tensor(out=hh[:], in0=tf_[:], scalar=-pi,
                                           in1=AR[:], op0=ALU.mult, op1=ALU.add)
            SN = hd.tile([128, 6 * NC_], dt.float32)
            nc.scalar.activation(out=SN[:], in_=hh[:], func=AF.Sin, scale=2.0)
            cosv = {nm: SN[:, NC_ * i:NC_ * (i + 1)]
                    for i, nm in enumerate("ABCDEF")}

            acc = hd.tile([128, NC_], dt.float32, tag="acc0")
            nc.vector.tensor_scalar(out=acc[:], in0=cosv["A"], scalar1=K["A"],
                                    scalar2=c0, op0=ALU.mult, op1=ALU.add)
            for i, nm in enumerate("BCDE"):
                acc2 = hd.tile([128, NC_], dt.float32, tag=f"acc{i+1}")
                eng = tt_eng[i % 2]
                eng.scalar_tensor_tensor(out=acc2[:], in0=cosv[nm][:],
                                         scalar=K[nm], in1=acc[:],
                                         op0=ALU.mult, op1=ALU.add)
                acc = acc2
            Yall = hd.tile([128, NC_], dt.float32, tag="Yall")
            nc.vector.scalar_tensor_tensor(out=Yall[:], in0=cosv["F"],
                                           scalar=K["F"], in1=acc[:],
                                           op0=ALU.mult, op1=ALU.add)

            # ---- final: out0=-ln(1+e^{1-2y}), out1=-ln(1+e^{2y-1})
            V = hd.tile([128, NC_], dt.float32)
            nc.scalar.activation(out=V[:], in_=Yall[:], func=AF.Exp,
                                 bias=CST[:, 8:9], scale=-2.0)      # e^{1-2y}
            Wr = hd.tile([128, NC_], dt.float32)
            nc.vector.reciprocal(out=Wr[:], in_=V[:])               # e^{2y-1}
            L0 = hd.tile([128, NC_], dt.float32)
            nc.scalar.activation(out=L0[:], in_=V[:], func=AF.Ln,
                                 bias=CST[:, 8:9], scale=1.0)       # ln(1+v)
            L1 = hd.tile([128, NC_], dt.float32)
            nc.scalar.activation(out=L1[:], in_=Wr[:], func=AF.Ln,
                                 bias=CST[:, 8:9], scale=1.0)
            O = hd.tile([128, 2 * NC_], dt.float32)
            Ov = O[:].rearrange("p (c u) -> p c u", c=2)
            nc.vector.tensor_scalar(out=Ov[:, 0, :], in0=L0[:], scalar1=-1.0,
                                    scalar2=None, op0=ALU.mult)
            nc.vector.tensor_scalar(out=Ov[:, 1, :], in0=L1[:], scalar1=-1.0,
                                    scalar2=None, op0=ALU.mult)
            # y layout [2, BC]: dst[c, it*NT + s*128 + p] <- O[p, (c, it, s)]
            for c in range(2):
                nc.sync.dma_start(
                    out=y_d[c, :].rearrange("(t s p) -> p t s", p=128, s=NSUB),
                    in_=Ov[:, c, :].rearrange("p (t s) -> p t s", s=NSUB))

    nc.compile()
    return nc


def kernel(x, conv1_w, conv1_b, conv2_w, conv2_b, fc1_w, fc1_b,
           fc2_w, fc2_b, fc3_w, fc3_b, qnn_params):
    x = np.asarray(x, dtype=np.float32).reshape(B, 784)
    a = build_a(x)
    W1 = build_w1(conv1_w)
    W2 = build_w2(conv2_w)
    WF1 = build_wfc1(fc1_w)
    WF2 = _bf16(np.asarray(fc2_w, np.float32).T)  # [64, 2]
    c0, K, b20, b21 = head_constants(qnn_params, fc3_w, fc3_b,
                                     np.asarray(fc2_b, np.float32))
    cst = np.zeros((128, 16), dtype=np.float32)
    b1 = np.asarray(conv1_b, np.float32)
    b2 = np.asarray(conv2_b, np.float32)
    for p in range(128):
        pay = p & 31
        cst[p, 0] = b1[pay & 1] if pay < 24 else 0.0   # conv1 bias (ch = p&1)
        cst[p, 1] = b2[pay >> 1]                        # conv2 bias (oc)
    cst[0:64, 2] = np.asarray(fc1_b, np.float32)
    cst[:, 8] = 1.0

    weights = {"head": (c0, K, b20, b21)}
    nc = build_program(weights)

    in_maps = []
    for c in range(NCORES):
        sl = slice(c * BC, (c + 1) * BC)
        in_maps.append({
            "a_c1": np.ascontiguousarray(a[:, :, sl]),
            "w1": W1, "w2": W2, "wf1": WF1, "wf2": WF2, "cst": cst,
        })
    res = run_bass_kernel_spmd(nc, in_maps, list(range(NCORES)),
                               trace=bool(int(os.environ.get("BASS_TRACE_KERNEL", "0"))))
    if res.exec_time_ns is not None:
        print(f"HW exec time: {res.exec_time_ns} ns")
    global LAST_RESULTS
    LAST_RESULTS = res.results
    out = np.empty((B, 2), dtype=np.float32)
    for c in range(NCORES):
        out[c * BC:(c + 1) * BC] = res.results[c]["y"].T
    return out
